# revision 5
# baseline (speedup 1.0000x reference)
# Trainium2 Bass kernel for nn_Div_15719580304337.
#
# Reference semantics (per element):
#   x2 = data2_q * data2_scale; sign = sign(x2); ax = |x2|
#   recip_q = piecewise-quantized reciprocal of ax via two 256-entry uniform-grid
#             LUTs (dense [0.01,1], sparse [1,7]) with saturating left constant
#             (right regions unreachable: max ax = 32768*2e-4 = 6.5536)
#   out = clip(round(data1_q*data1_scale * recip_q*TABLE_SCALE / out_scale), -32768, 32767)
#
# Because the LUTs are quant(1/x) on uniform grids, the gather is replaced by
# snap-to-grid + hardware reciprocal:  idx = round((ax-xmin)*r); y = 1/(c1*idx+c0)
# (the affine runs inside the ACT Reciprocal's scale/bias FMA); val = min(round(y),
# 32767).  All index/val computations were verified bit-exact against the jax
# reference for every possible data2_q value (65536-point sweep on hardware).
#
# Sharding: fully elementwise; the flattened 64Mi elements are split into 8
# contiguous 8Mi chunks, one per NeuronCore; no communication.
import os
import numpy as np

f32 = np.float32
f64 = np.float64

# ---- fixed problem constants (from the nn.Module, not the inputs) ----
TS_F64 = 2.0 / 0.01 / 65535.0        # TABLE_SCALE
M = 12582912.0                        # 1.5 * 2^23 fp32 round-to-int magic
R_D = float(f32(f64(255.0) / f64(0.99)))          # dense 1/step
R_S = float(f32(42.5))                            # sparse 1/step
BS_FMA = float(f32(-42.5))                        # t_s = fma(ax, R_S, -42.5)
C1D = float(f32((f64(0.99) / 255.0) * TS_F64))    # dense grid*TS slope
C0D = float(f32(f64(0.01) * TS_F64))              # dense grid*TS origin
C1S = float(f32((f64(6.0) / 255.0) * TS_F64))
C0S = float(f32(f64(1.0) * TS_F64))

N_CORES = 8
SHAPE = (4, 16, 1024, 1024)
TOTAL = 4 * 16 * 1024 * 1024
PER_CORE = TOTAL // N_CORES          # 8388608
P = 128
F = 2048
T = PER_CORE // (P * F)              # 32 tiles

_cached = {}


def _register_custom_ops():
    from concourse.dve_spec import (
        Spec, Src0, Src1, C0, C1, C2, Zero, maxx, minn, select, lower,
        _has_src1 as has_src1,
    )
    from concourse import dve_ops as DOPS
    from concourse.dve_uop import DveOpSpec

    def _r32(x):
        return np.asarray(x, np.float64).astype(np.float32)

    def _ref_dense(in0, in1, c0, c1, c2):
        a = in0.astype(f32)
        s = _r32(_r32(a.astype(f64) + f32(c0)).astype(f64) * f32(c1))
        s = np.maximum(s, f32(0.0))
        s = _r32(s.astype(f64) + f32(c2))
        return _r32(s.astype(f64) - f32(c2))

    def _ref_post(in0, in1, c0, c1, c2):
        y = in0.astype(f32)
        v = _r32(y.astype(f64) + f32(c0))
        v = _r32(v.astype(f64) - f32(c0))
        v = np.minimum(v, f32(c1))
        v2 = _r32(v.astype(f64) * f32(c2))
        return np.where(in1.astype(f32) < 0, -v2, v2)

    def _ref_final(in0, in1, c0, c1, c2):
        q = _r32(in0.astype(f32).astype(f64) * in1.astype(f32))
        r = _r32(q.astype(f64) + f32(c0))
        r = _r32(r.astype(f64) - f32(c0))
        return np.maximum(np.minimum(r, f32(c1)), f32(c2))

    def _reg(name, spec):
        for op in DOPS.OPS:
            if op.name == name:
                return op
        row = DOPS._CUSTOM_DVE_ROW_BASE + len(DOPS.OPS)
        assert row < 0x20, "custom DVE rows exhausted"
        shas = {}
        for ver in ("v3", "v4"):
            tmp = DveOpSpec(name=name, opcode=row, uops=lower(spec, ver=ver),
                            rd1_en=has_src1(spec))
            shas[ver] = tmp.sha(ver)
        op = DOPS.DveOp(name, spec, subdim=False, uops_sha=shas)
        DOPS.OPS.append(op)
        DOPS._SUB_OPCODE_FOR_NAME[name] = row
        DOPS.CUSTOM_DVE_SPECS[name] = spec
        return op

    dense = _reg("DIV_DENSE_IDX", Spec(
        body=(maxx((Src0 + C0) * C1, Zero) + C2) - C2,
        reference=_ref_dense))
    post = _reg("DIV_POST", Spec(
        body=select(Src1 < Zero, Zero - (minn((Src0 + C0) - C0, C1) * C2),
                    minn((Src0 + C0) - C0, C1) * C2),
        reference=_ref_post))
    final = _reg("DIV_FINAL", Spec(
        body=maxx(minn(((Src0 * Src1) + C0) - C0, C1), C2),
        reference=_ref_final))
    return dense, post, final


def _act_manual(nc, out, in_, func, bias=0.0, scale=1.0):
    import concourse.mybir as mybir
    eng = nc.scalar
    ins = [eng.lower_ap(in_)]
    for arg in (bias, scale, 0.0):
        ins.append(mybir.ImmediateValue(dtype=mybir.dt.float32, value=float(arg)))
    return eng.add_instruction(mybir.InstActivation(
        name=nc.get_next_instruction_name(), func=func,
        ins=ins, outs=[eng.lower_ap(out)]))


def _build_program(s2: float, tsf: float):
    import concourse.bacc as bacc
    import concourse.mybir as mybir
    import concourse.tile as tile

    A = mybir.AluOpType
    AF = mybir.ActivationFunctionType
    dt = mybir.dt
    DENSE_OP, POST_OP, FINAL_OP = _register_custom_ops()

    nc = bacc.Bacc("TRN2", target_bir_lowering=False, debug=False,
                   num_devices=N_CORES)
    t1_d = nc.dram_tensor("t1", [T, P, F], dt.int32, kind="ExternalInput").ap()
    t2_d = nc.dram_tensor("t2", [T, P, F], dt.int32, kind="ExternalInput").ap()
    out_d = nc.dram_tensor("out", [T, P, F], dt.float32, kind="ExternalOutput").ap()

    with tile.TileContext(nc) as tc:
        with tc.tile_pool(name="io", bufs=3) as io, \
             tc.tile_pool(name="tmp", bufs=2) as tmp:
            for t in range(T):
                t1t = io.tile([P, F], dt.int32, tag="t1")
                nc.sync.dma_start(t1t[:], t1_d[t])
                t2t = io.tile([P, F], dt.int32, tag="t2")
                nc.sync.dma_start(t2t[:], t2_d[t])

                ax = tmp.tile([P, F], dt.float32, tag="ax")
                nc.scalar.activation(ax[:], t2t[:], AF.Abs, bias=0.0, scale=float(s2))
                t_s = tmp.tile([P, F], dt.float32, tag="t_s")
                nc.scalar.activation(t_s[:], ax[:], AF.Copy, bias=BS_FMA, scale=R_S)
                mask = tmp.tile([P, F], dt.uint32, tag="mask")
                nc.scalar.activation(mask[:], t_s[:], AF.Relu, bias=0.0, scale=1e6)
                idx_d = tmp.tile([P, F], dt.float32, tag="idx_d")
                nc.vector._custom_dve(DENSE_OP, out=idx_d[:], in0=ax[:],
                                      s0=-0.01, s1=R_D, imm2=M)
                nc.vector.tensor_scalar(t_s[:], t_s[:], M, -M, A.add, A.add)
                y = tmp.tile([P, F], dt.float32, tag="y")
                _act_manual(nc, y[:], idx_d[:], AF.Reciprocal, bias=C0D, scale=C1D)
                y_s = tmp.tile([P, F], dt.float32, tag="y_s")
                _act_manual(nc, y_s[:], t_s[:], AF.Reciprocal, bias=C0S, scale=C1S)
                nc.vector.copy_predicated(y[:], mask[:], y_s[:])
                nc.vector._custom_dve(POST_OP, out=y[:], in0=y[:], in1=t2t[:],
                                      s0=M, s1=32767.0, imm2=tsf)
                outt = io.tile([P, F], dt.float32, tag="out")
                nc.vector._custom_dve(FINAL_OP, out=outt[:], in0=t1t[:], in1=y[:],
                                      s0=M, s1=32767.0, imm2=-32768.0)
                nc.sync.dma_start(out_d[t], outt[:])
    nc.compile()
    return nc


def _make_runner(nc):
    """jit(shard_map(...)) over 8 cores for the prebuilt Bass module.

    Returns (sharded_fn, out_shape, out_dtype). Call as
    sharded_fn(t1_global, t2_global, zeros_global) with arrays whose axis 0 is
    N_CORES*T; the zeros argument is donated as the output buffer.
    """
    import jax
    import concourse.mybir as mybir
    from jax.experimental.shard_map import shard_map
    from jax.sharding import Mesh, PartitionSpec
    from concourse.bass2jax import (
        _bass_exec_p, install_neuronx_cc_hook, partition_id_tensor,
    )

    install_neuronx_cc_hook()

    in_names = ["t1", "t2"]
    out_names = ["out"]
    all_names = in_names + out_names
    if nc.partition_id_tensor is not None:
        all_names = all_names + [nc.partition_id_tensor.name]
    out_avals = [jax.core.ShapedArray((T, P, F), np.float32)]

    def _body(*args):
        operands = list(args)
        if nc.partition_id_tensor is not None:
            operands.append(partition_id_tensor())
        outs = _bass_exec_p.bind(
            *operands,
            out_avals=tuple(out_avals),
            in_names=tuple(all_names),
            out_names=tuple(out_names),
            lowering_input_output_aliases=(),
            sim_require_finite=True,
            sim_require_nnan=True,
            nc=nc,
        )
        return tuple(outs)

    devices = jax.devices()[:N_CORES]
    assert len(devices) == N_CORES
    mesh = Mesh(np.asarray(devices), ("core",))
    sharded = jax.jit(
        shard_map(_body, mesh=mesh,
                  in_specs=(PartitionSpec("core"),) * 3,
                  out_specs=(PartitionSpec("core"),),
                  check_rep=False),
        donate_argnums=(2,), keep_unused=True,
    )
    return sharded


def _get_runner(s2: float, tsf: float):
    key = (s2, tsf)
    if key not in _cached:
        nc = _build_program(s2, tsf)
        _cached[key] = _make_runner(nc)
    return _cached[key]


def kernel(**inputs) -> np.ndarray:
    d1 = np.ascontiguousarray(np.asarray(inputs["data1_q"], dtype=np.int32))
    d2 = np.ascontiguousarray(np.asarray(inputs["data2_q"], dtype=np.int32))
    s1 = float(np.asarray(inputs["data1_scale"], dtype=np.float32).reshape(-1)[0])
    s2 = float(np.asarray(inputs["data2_scale"], dtype=np.float32).reshape(-1)[0])
    out_s = float(np.asarray(inputs["out_scale"], dtype=np.float32).reshape(-1)[0])
    assert d1.shape == SHAPE and d2.shape == SHAPE

    tsf = float(f32(TS_F64 * f64(s1) / f64(out_s)))
    sharded = _get_runner(s2, tsf)

    t1g = d1.reshape(N_CORES * T, P, F)
    t2g = d2.reshape(N_CORES * T, P, F)
    zeros = np.zeros((N_CORES * T, P, F), np.float32)
    (outg,) = sharded(t1g, t2g, zeros)
    # Assemble from per-device shards (a direct np.asarray of the global
    # sharded array is not supported on this backend).
    out = np.empty((N_CORES * T, P, F), np.float32)
    for shard in outg.addressable_shards:
        idx = shard.index
        out[idx] = np.asarray(shard.data)
    return out.reshape(SHAPE)
